# revision 1
# baseline (speedup 1.0000x reference)
"""Trainium2 Bass kernel for AttentionalAggregation-style GNN pooling.

reference math:
    enc  = relu(lane_encoding @ W.T + b)            # [M=400000, 512]
    maxp = segment_max(enc, seg)                    # [N=25000, 512], 16 lanes/group
    avgp = segment_mean(enc, seg)                   # [N=25000, 512]
    out  = concat([maxp, avgp], axis=1)             # [N, 1024]

Strategy (8 NeuronCores, data-parallel over lanes; each core owns whole groups):
  - Host pre-transposes x -> XT [128, M] so the contraction dim (features) is
    the SBUF partition dim for both matmul operands; no on-device transposes.
  - Matmul emits encT tiles [128 outdims, lanes] in PSUM. bf16 3-term
    compensated split (x = xh + xl, W = wh + wl; xh@wh + xh@wl + xl@wh) runs
    at full PE rate with ~1e-5 relative accuracy.
  - relu(u + b) is fused into the mandatory PSUM->SBUF copy on the Scalar
    engine: activation(Relu, scale=1/16, bias=b/16) (bias is per-partition =
    per-outdim in this layout). The 1/16 folds the mean-pool divide in.
  - Pooling over 16 contiguous lanes = windowed reduce along the free dim on
    the Vector engine ([128, G, 16] -> [128, G]); max pool needs a x16 fixup
    since relu(max(u)+b) = 16 * max_l relu((u+b)/16).
  - Pooled outputs stay transposed ([512, G]); host reassembles [N, 1024].
"""
import sys

sys.path.insert(0, "/opt/trn_rl_repo")

import numpy as np
import ml_dtypes

import concourse.bass as bass
import concourse.bacc as bacc
import concourse.tile as tile
from concourse import mybir
from concourse.bass_utils import run_bass_kernel_spmd

N_CORES = 8
IN_DIM = 128
OUT_DIM = 512
N_OBS = 25000
M_LANES = 400000
GS = 16                       # lanes per group
M_C = M_LANES // N_CORES      # 50000 lanes per core
G_C = N_OBS // N_CORES        # 3125 groups per core
N_CHUNK = OUT_DIM // 128      # 4 outdim chunks
BLK = 2048                    # lanes per DMA/compute block (4 psum banks)

MODE = "bf16x3"               # "bf16x3" | "f32r" | "f32"
# Fraction of sum-pool spans offloaded from the Vector engine to a GPSIMD
# tensor_tensor add-chain (DVE is the bottleneck; gpsimd TT is its fast path).
GPS_SUM_TENTHS = 7            # fraction (in tenths) of sum-pool spans on gpsimd            # 0..10

_compiled = {}


def _build(mode: str) -> bass.Bass:
    nc = bacc.Bacc(None, target_bir_lowering=False)
    f32 = mybir.dt.float32

    if mode == "bf16x3":
        bf16 = mybir.dt.bfloat16
        xth_d = nc.dram_tensor("xth", [IN_DIM, M_C], bf16, kind="ExternalInput")
        xtl_d = nc.dram_tensor("xtl", [IN_DIM, M_C], bf16, kind="ExternalInput")
        wth_d = nc.dram_tensor("wth", [IN_DIM, OUT_DIM], bf16, kind="ExternalInput")
        wtl_d = nc.dram_tensor("wtl", [IN_DIM, OUT_DIM], bf16, kind="ExternalInput")
    else:
        mmdt = mybir.dt.float32r if mode == "f32r" else mybir.dt.float32
        xt_d = nc.dram_tensor("xt", [IN_DIM, M_C], mmdt, kind="ExternalInput")
        wt_d = nc.dram_tensor("wt", [IN_DIM, OUT_DIM], mmdt, kind="ExternalInput")
    bsc_d = nc.dram_tensor("bsc", [128, N_CHUNK], f32, kind="ExternalInput")
    omax_d = nc.dram_tensor("omax", [OUT_DIM, G_C], f32, kind="ExternalOutput")
    osum_d = nc.dram_tensor("osum", [OUT_DIM, G_C], f32, kind="ExternalOutput")

    n_blk = (M_C + BLK - 1) // BLK

    with tile.TileContext(nc) as tc:
        with (
            tc.tile_pool(name="singles", bufs=1) as singles,
            tc.tile_pool(name="xin", bufs=3) as xin,
            tc.tile_pool(name="rsb", bufs=4) as rsb,
            tc.tile_pool(name="acc", bufs=1) as accp,
            tc.tile_pool(name="gtmp", bufs=4) as gtmp,
            tc.tile_pool(name="psum", bufs=2, space="PSUM") as psum,
        ):
            # resident weights / bias
            if mode == "bf16x3":
                wth_sb = singles.tile([IN_DIM, OUT_DIM], mybir.dt.bfloat16)
                nc.sync.dma_start(out=wth_sb, in_=wth_d[:, :])
                wtl_sb = singles.tile([IN_DIM, OUT_DIM], mybir.dt.bfloat16)
                nc.sync.dma_start(out=wtl_sb, in_=wtl_d[:, :])
            else:
                wt_sb = singles.tile(
                    [IN_DIM, OUT_DIM],
                    mybir.dt.float32r if mode == "f32r" else f32,
                )
                nc.sync.dma_start(out=wt_sb, in_=wt_d[:, :])
            bsc_sb = singles.tile([128, N_CHUNK], f32)
            nc.sync.dma_start(out=bsc_sb, in_=bsc_d[:, :])

            # persistent pooled accumulators [128, chunk, G_C]
            maxp_sb = accp.tile([128, N_CHUNK, G_C], f32)
            sump_sb = accp.tile([128, N_CHUNK, G_C], f32)

            # prime the ACT spline-table load while the first DMA is in flight
            warm_sb = singles.tile([128, 2], f32)
            nc.vector.memset(warm_sb, 0.0)
            nc.scalar.activation(
                out=warm_sb, in_=warm_sb,
                func=mybir.ActivationFunctionType.Relu, bias=0.0, scale=1.0,
            )

            # block lane-ranges: small first block primes the pipeline fast
            starts = [0, 512]
            while starts[-1] + BLK < M_C:
                starts.append(starts[-1] + BLK)
            blocks = [(s, min(s + BLK, M_C) - s if i == len(starts) - 1
                       else (starts[i + 1] - s))
                      for i, s in enumerate(starts)]
            blocks = [(s, min(e, M_C - s)) for s, e in blocks]
            flush_after = {blocks[min(k, len(blocks) - 1)][0]
                           for k in (7, 13, 19, 23, len(blocks) - 1)}
            flush_from = 0

            for ib, (l0, lb) in enumerate(blocks):
                gb = lb // GS
                g0 = l0 // GS

                if mode == "bf16x3":
                    xth_sb = xin.tile([IN_DIM, BLK], mybir.dt.bfloat16, tag="xth")
                    nc.sync.dma_start(out=xth_sb[:, :lb], in_=xth_d[:, l0 : l0 + lb])
                    xtl_sb = xin.tile([IN_DIM, BLK], mybir.dt.bfloat16, tag="xtl")
                    nc.sync.dma_start(out=xtl_sb[:, :lb], in_=xtl_d[:, l0 : l0 + lb])
                else:
                    xt_sb = xin.tile(
                        [IN_DIM, BLK],
                        mybir.dt.float32r if mode == "f32r" else f32,
                        tag="xt",
                    )
                    nc.sync.dma_start(out=xt_sb[:, :lb], in_=xt_d[:, l0 : l0 + lb])

                n_wave = (lb + 511) // 512
                for c in range(N_CHUNK):
                    enc_ps = psum.tile([128, BLK], f32, tag="enc")
                    for w in range(n_wave):
                        w0 = w * 512
                        lw = min(512, lb - w0)
                        out_ap = enc_ps[:, w0 : w0 + lw]
                        if mode == "bf16x3":
                            nc.tensor.matmul(
                                out_ap,
                                wth_sb[:, c * 128 : (c + 1) * 128],
                                xth_sb[:, w0 : w0 + lw],
                                start=True, stop=False,
                            )
                            nc.tensor.matmul(
                                out_ap,
                                wtl_sb[:, c * 128 : (c + 1) * 128],
                                xth_sb[:, w0 : w0 + lw],
                                start=False, stop=False,
                            )
                            nc.tensor.matmul(
                                out_ap,
                                wth_sb[:, c * 128 : (c + 1) * 128],
                                xtl_sb[:, w0 : w0 + lw],
                                start=False, stop=True,
                            )
                        else:
                            nc.tensor.matmul(
                                out_ap,
                                wt_sb[:, c * 128 : (c + 1) * 128],
                                xt_sb[:, w0 : w0 + lw],
                                start=True, stop=True,
                            )

                    # r = relu(u/16 + b/16)  (PSUM -> SBUF, bias per partition)
                    r_sb = rsb.tile([128, BLK], f32, tag="r")
                    nc.scalar.activation(
                        out=r_sb[:, :lb],
                        in_=enc_ps[:, :lb],
                        func=mybir.ActivationFunctionType.Relu,
                        bias=bsc_sb[:, c : c + 1],
                        scale=1.0 / GS,
                    )
                    r3 = r_sb[:, :lb].rearrange("p (g s) -> p g s", s=GS)
                    nc.vector.reduce_max(
                        out=maxp_sb[:, c, g0 : g0 + gb],
                        in_=r3,
                        axis=mybir.AxisListType.X,
                    )
                    span_idx = ib * N_CHUNK + c
                    if span_idx % 10 < GPS_SUM_TENTHS:
                        # sum pool on gpsimd: 4-op binary tree. Reads r only
                        # in the first op so the r tile releases quickly.
                        t1 = gtmp.tile([128, gb, 8], f32, tag="t1")
                        nc.gpsimd.tensor_tensor(
                            out=t1, in0=r3[:, :, 0::2], in1=r3[:, :, 1::2],
                            op=mybir.AluOpType.add,
                        )
                        t2 = gtmp.tile([128, gb, 4], f32, tag="t2")
                        nc.gpsimd.tensor_tensor(
                            out=t2, in0=t1[:, :, 0::2], in1=t1[:, :, 1::2],
                            op=mybir.AluOpType.add,
                        )
                        t3 = gtmp.tile([128, gb, 2], f32, tag="t3")
                        nc.gpsimd.tensor_tensor(
                            out=t3, in0=t2[:, :, 0::2], in1=t2[:, :, 1::2],
                            op=mybir.AluOpType.add,
                        )
                        nc.gpsimd.tensor_tensor(
                            out=sump_sb[:, c, g0 : g0 + gb],
                            in0=t3[:, :, 0], in1=t3[:, :, 1],
                            op=mybir.AluOpType.add,
                        )
                    else:
                        nc.vector.reduce_sum(
                            out=sump_sb[:, c, g0 : g0 + gb],
                            in_=r3,
                            axis=mybir.AxisListType.X,
                        )

                # Stream fixup + output DMA for finished group ranges so the
                # kernel tail overlaps with compute.
                # max pool fixup: relu(max_l u + b) = 16 * max_l relu((u+b)/16)
                if l0 in flush_after:
                    r0, r1 = flush_from, g0 + gb
                    flush_from = r1
                    nc.scalar.mul(
                        out=maxp_sb[:, :, r0:r1],
                        in_=maxp_sb[:, :, r0:r1],
                        mul=float(GS),
                    )
                    for c in range(N_CHUNK):
                        nc.sync.dma_start(
                            out=omax_d[c * 128 : (c + 1) * 128, r0:r1],
                            in_=maxp_sb[:, c, r0:r1],
                        )
                        nc.sync.dma_start(
                            out=osum_d[c * 128 : (c + 1) * 128, r0:r1],
                            in_=sump_sb[:, c, r0:r1],
                        )

    nc.compile()
    return nc


def _get_nc(mode: str) -> bass.Bass:
    if mode not in _compiled:
        _compiled[mode] = _build(mode)
    return _compiled[mode]


def _host_prep(lane_encoding, W, b, mode: str):
    """Returns the per-core in_maps."""
    xT = np.ascontiguousarray(lane_encoding.T)          # [128, M]
    wT = np.ascontiguousarray(W.T)                      # [128, 512]
    # bias, pre-divided by GS, in [128, chunk] layout
    bsc = np.ascontiguousarray(
        (b.reshape(N_CHUNK, 128).T / GS).astype(np.float32)
    )

    in_maps = []
    if mode == "bf16x3":
        bf = ml_dtypes.bfloat16
        xh = xT.astype(bf)
        xl = (xT - xh.astype(np.float32)).astype(bf)
        wh = wT.astype(bf)
        wl = (wT - wh.astype(np.float32)).astype(bf)
        for c in range(N_CORES):
            sl = slice(c * M_C, (c + 1) * M_C)
            in_maps.append({
                "xth": np.ascontiguousarray(xh[:, sl]),
                "xtl": np.ascontiguousarray(xl[:, sl]),
                "wth": wh, "wtl": wl, "bsc": bsc,
            })
    else:
        for c in range(N_CORES):
            sl = slice(c * M_C, (c + 1) * M_C)
            in_maps.append({
                "xt": np.ascontiguousarray(xT[:, sl]),
                "wt": wT, "bsc": bsc,
            })
    return in_maps


def _run(lane_encoding, W, b, mode: str, trace: bool = False):
    nc = _get_nc(mode)
    in_maps = _host_prep(lane_encoding, W, b, mode)
    try:
        res = run_bass_kernel_spmd(
            nc, in_maps, core_ids=list(range(N_CORES)), trace=trace
        )
    except Exception:
        # transient NRT_EXEC_UNIT_UNRECOVERABLE wedges have been observed;
        # a single retry usually succeeds
        res = run_bass_kernel_spmd(
            nc, in_maps, core_ids=list(range(N_CORES)), trace=trace
        )
    out = np.empty((N_OBS, 2 * OUT_DIM), dtype=np.float32)
    for c in range(N_CORES):
        gsl = slice(c * G_C, (c + 1) * G_C)
        out[gsl, :OUT_DIM] = res.results[c]["omax"].T
        out[gsl, OUT_DIM:] = res.results[c]["osum"].T
    return out, res


def kernel(obs_encoding, lane_encoding, same_obs_mask, W, b):
    out, _ = _run(
        np.asarray(lane_encoding, dtype=np.float32),
        np.asarray(W, dtype=np.float32),
        np.asarray(b, dtype=np.float32),
        MODE,
    )
    return out



# revision 4
# speedup vs baseline: 1.5639x; 1.5639x over previous
"""Trainium2 Bass kernel for AttentionalAggregation-style GNN pooling.

reference math:
    enc  = relu(lane_encoding @ W.T + b)            # [M=400000, 512]
    maxp = segment_max(enc, seg)                    # [N=25000, 512], 16 lanes/group
    avgp = segment_mean(enc, seg)                   # [N=25000, 512]
    out  = concat([maxp, avgp], axis=1)             # [N, 1024]

Strategy (8 NeuronCores, data-parallel over lanes; each core owns whole groups):
  - Host pre-transposes x -> XT [128, M] (bf16) with an "s-major" column
    permutation inside each 2048-lane block: column s*G + g holds lane s of
    group g.  Pooling over a group then becomes a pairwise halving tree over
    CONTIGUOUS slabs, which runs on the Vector engine in 2x bf16 mode
    (599ns per 1024-out TT vs 2193ns for a 1x windowed reduce).
  - Single bf16 matmul per 512-col wave (PSUM f32 accumulate).  The 2e-2
    rel-err budget makes the bf16x3 compensated split unnecessary.
  - ACT drains PSUM with fused relu(u + b) -> bf16 r-tiles (1888ns/2048).
  - DVE runs max and sum trees on the r-tiles, batched across the 4 outdim
    chunks of a block to amortize per-op overhead.  No GPSIMD: its shared
    SBUF port fully serializes against DVE 2-port TT ops (measured).
  - Outputs stay transposed bf16 [512, G]; host converts / divides by 16.
"""
import sys

sys.path.insert(0, "/opt/trn_rl_repo")

import numpy as np
import ml_dtypes

import concourse.bass as bass
import concourse.bacc as bacc
import concourse.tile as tile
from concourse import mybir
from concourse.bass_utils import run_bass_kernel_spmd

N_CORES = 8
IN_DIM = 128
OUT_DIM = 512
N_OBS = 25000
M_LANES = 400000
GS = 16                       # lanes per group
M_C = M_LANES // N_CORES      # 50000 lanes per core
G_C = N_OBS // N_CORES        # 3125 groups per core
N_CHUNK = OUT_DIM // 128      # 4 outdim chunks
BLK = 2048                    # lanes per block (4 psum banks)

MODE = "bf16tree"

_compiled = {}


def _tree(nc, rblk, dst, gb, op, tpool):
    """Halving tree over the 16 s-slabs of rblk [128, 4, 16*gb] -> dst
    [128, 4, gb].  All levels contiguous-slab TT ops (bf16 2x mode)."""
    bf16 = mybir.dt.bfloat16
    cur = rblk
    for lvl, w in enumerate((8 * gb, 4 * gb, 2 * gb, gb)):
        last = w == gb
        nxt = dst if last else tpool.tile([128, N_CHUNK, w], bf16, tag=f"t{lvl}")
        nc.vector.tensor_tensor(
            out=nxt if last else nxt[:, :, 0:w],
            in0=cur[:, :, 0:w],
            in1=cur[:, :, w : 2 * w],
            op=op,
        )
        cur = nxt


def _build(mode: str) -> bass.Bass:
    nc = bacc.Bacc(None, target_bir_lowering=False)
    f32 = mybir.dt.float32
    bf16 = mybir.dt.bfloat16

    xth_d = nc.dram_tensor("xth", [IN_DIM, M_C], bf16, kind="ExternalInput")
    wth_d = nc.dram_tensor("wth", [IN_DIM, OUT_DIM], bf16, kind="ExternalInput")
    bsc_d = nc.dram_tensor("bsc", [128, N_CHUNK], f32, kind="ExternalInput")
    omax_d = nc.dram_tensor("omax", [OUT_DIM, G_C], bf16, kind="ExternalOutput")
    osum_d = nc.dram_tensor("osum", [OUT_DIM, G_C], bf16, kind="ExternalOutput")

    n_blk = (M_C + BLK - 1) // BLK          # 25 (24 full + tail 848)

    with tile.TileContext(nc) as tc:
        with (
            tc.tile_pool(name="singles", bufs=1) as singles,
            tc.tile_pool(name="xin", bufs=3) as xin,
            tc.tile_pool(name="rblk", bufs=2) as rpool,
            tc.tile_pool(name="trees", bufs=2) as tpool,
            tc.tile_pool(name="acc", bufs=1) as accp,
            tc.tile_pool(name="psum", bufs=2, space="PSUM") as psum,
        ):
            wth_sb = singles.tile([IN_DIM, OUT_DIM], bf16)
            nc.sync.dma_start(out=wth_sb, in_=wth_d[:, :])
            bsc_sb = singles.tile([128, N_CHUNK], f32)
            nc.sync.dma_start(out=bsc_sb, in_=bsc_d[:, :])

            # pooled accumulators [128, chunk, G_C] bf16
            maxp_sb = accp.tile([128, N_CHUNK, G_C], bf16)
            sump_sb = accp.tile([128, N_CHUNK, G_C], bf16)

            # prime ACT spline table before the pipeline starts
            warm_sb = singles.tile([128, 2], f32)
            nc.vector.memset(warm_sb, 0.0)
            nc.scalar.activation(
                out=warm_sb, in_=warm_sb,
                func=mybir.ActivationFunctionType.Relu, bias=0.0, scale=1.0,
            )

            flush_every = 5
            flush_from = 0

            for ib in range(n_blk):
                l0 = ib * BLK
                lb = min(BLK, M_C - l0)
                gb = lb // GS
                g0 = l0 // GS

                xt_sb = xin.tile([IN_DIM, BLK], bf16, tag="xt")
                nc.sync.dma_start(out=xt_sb[:, :lb], in_=xth_d[:, l0 : l0 + lb])

                r_sb = rpool.tile([128, N_CHUNK, BLK], bf16, tag="r")

                n_wave = (lb + 511) // 512
                for c in range(N_CHUNK):
                    enc_ps = psum.tile([128, BLK], f32, tag="enc")
                    for w in range(n_wave):
                        w0 = w * 512
                        lw = min(512, lb - w0)
                        nc.tensor.matmul(
                            enc_ps[:, w0 : w0 + lw],
                            wth_sb[:, c * 128 : (c + 1) * 128],
                            xt_sb[:, w0 : w0 + lw],
                            start=True, stop=True,
                        )
                    # r = relu(u + b): fused PSUM->SBUF drain on ACT
                    nc.scalar.activation(
                        out=r_sb[:, c, :lb],
                        in_=enc_ps[:, :lb],
                        func=mybir.ActivationFunctionType.Relu,
                        bias=bsc_sb[:, c : c + 1],
                        scale=1.0,
                    )

                # pooling trees over the whole block (4 chunks at once)
                r3 = r_sb[:, :, :lb]
                _tree(nc, r3, maxp_sb[:, :, g0 : g0 + gb], gb,
                      mybir.AluOpType.max, tpool)
                _tree(nc, r3, sump_sb[:, :, g0 : g0 + gb], gb,
                      mybir.AluOpType.add, tpool)

                if (ib + 1) % flush_every == 0 or ib == n_blk - 1:
                    r0, r1 = flush_from, g0 + gb
                    flush_from = r1
                    for c in range(N_CHUNK):
                        nc.sync.dma_start(
                            out=omax_d[c * 128 : (c + 1) * 128, r0:r1],
                            in_=maxp_sb[:, c, r0:r1],
                        )
                        nc.sync.dma_start(
                            out=osum_d[c * 128 : (c + 1) * 128, r0:r1],
                            in_=sump_sb[:, c, r0:r1],
                        )

    nc.compile()
    return nc


def _get_nc(mode: str) -> bass.Bass:
    if mode not in _compiled:
        _compiled[mode] = _build(mode)
    return _compiled[mode]


def _host_prep(lane_encoding, W, b, mode: str):
    """Per-core in_maps.  x is transposed, bf16-cast, and column-permuted to
    s-major inside each 2048-lane block."""
    bf = ml_dtypes.bfloat16
    xT = np.ascontiguousarray(lane_encoding.T).astype(bf)   # [128, M]
    wT = np.ascontiguousarray(W.T).astype(bf)               # [128, 512]
    bsc = np.ascontiguousarray(b.reshape(N_CHUNK, 128).T.astype(np.float32))

    n_full = M_C // BLK                 # full 2048 blocks per core
    tail = M_C - n_full * BLK

    in_maps = []
    for c in range(N_CORES):
        xc = xT[:, c * M_C : (c + 1) * M_C]
        main = xc[:, : n_full * BLK].reshape(IN_DIM, n_full, BLK // GS, GS)
        main = main.transpose(0, 1, 3, 2).reshape(IN_DIM, n_full * BLK)
        parts = [main]
        if tail:
            tl = xc[:, n_full * BLK :].reshape(IN_DIM, tail // GS, GS)
            parts.append(tl.transpose(0, 2, 1).reshape(IN_DIM, tail))
        xs = np.ascontiguousarray(np.concatenate(parts, axis=1))
        in_maps.append({"xth": xs, "wth": wT, "bsc": bsc})
    return in_maps


def _run(lane_encoding, W, b, mode: str, trace: bool = False):
    nc = _get_nc(mode)
    in_maps = _host_prep(lane_encoding, W, b, mode)
    try:
        res = run_bass_kernel_spmd(
            nc, in_maps, core_ids=list(range(N_CORES)), trace=trace
        )
    except Exception:
        # transient NRT_EXEC_UNIT_UNRECOVERABLE wedges; one retry usually works
        res = run_bass_kernel_spmd(
            nc, in_maps, core_ids=list(range(N_CORES)), trace=trace
        )
    out = np.empty((N_OBS, 2 * OUT_DIM), dtype=np.float32)
    for c in range(N_CORES):
        gsl = slice(c * G_C, (c + 1) * G_C)
        out[gsl, :OUT_DIM] = res.results[c]["omax"].T.astype(np.float32)
        out[gsl, OUT_DIM:] = res.results[c]["osum"].T.astype(np.float32) / GS
    return out, res


def kernel(obs_encoding, lane_encoding, same_obs_mask, W, b):
    out, _ = _run(
        np.asarray(lane_encoding, dtype=np.float32),
        np.asarray(W, dtype=np.float32),
        np.asarray(b, dtype=np.float32),
        MODE,
    )
    return out
